# revision 8
# baseline (speedup 1.0000x reference)
"""Trainium2 Bass kernel for the show-attend-tell decoder problem.

Strategy: data-parallel over batch (B=128 -> 16 samples on each of 8 cores),
decoder weights replicated.  The sequential 51-step LSTM+attention loop runs
on-device; everything that does not feed the recurrence (embedding gate
contributions, lh/lz projections, the fc vocab projection) is hoisted out of
the loop and batched over (batch, time).

Host work is limited to index/layout prep: length sort, embedding gather,
transposes, tiny h0/c0 init matmuls, and alpha normalization (exp/sums are
produced on device).
"""

import numpy as np
import ml_dtypes

# ---------------------------------------------------------------- constants
B, P, ENC = 128, 196, 2048
A = E = H = 512
V = 10000
MAXLEN = 52
T = MAXLEN - 1            # 51 decode steps
NCORES = 8
Bc = B // NCORES          # 16 samples per core
BT = Bc * T               # 816 (bt rows, t-major: row = t*Bc + b)
BP = Bc * P               # 3136 ((b,p) rows, b-major: row = b*P + p)
KP = (BP + 127) // 128    # 25 k-tiles over the (b,p) axis
H4 = 4 * H                # 2048 gate width

BF16 = ml_dtypes.bfloat16

_CACHE = {}
TRACE = False
LAST_RESULT = None


# ------------------------------------------------------------------ patches
def _install_tile_patches():
    """Two workarounds for the 2026-05-04 walrus in this container:
    1) the Tile end-of-context drain carries one semaphore wait per logical
       processor; walrus rejects >2 sync waits on one SP CTRL instruction.
    2) ordinary Tile-scheduled instructions can also accumulate 3+ waits.
    Both are fixed by moving waits onto single-wait NOPs that precede the
    instruction on the same engine (engines execute in program order)."""
    import concourse.tile as tile
    from concourse.vector_clock import ScopedClock, VectorClock

    if getattr(tile.TileContext, "_drain_patch_installed", False):
        return

    def _patched_drain_and_barrier(self, tick_clock, wait_clock):
        nc = self.nc
        gc = tick_clock.global_clock
        ticks = eval(repr(gc).replace("VectorClock(", "").rstrip(")"))
        n = len(ticks)
        for p, tval in enumerate(ticks):
            if tval <= 0:
                continue
            clock_p = VectorClock([tval if i == p else 0 for i in range(n)])
            nop = nc.sync.nop(nofuse=True)
            wait_clock.add_sem_waits(nop.ins, ScopedClock({None: clock_p}))
        nc.sync.drain()
        nc.all_engine_barrier()
        assert self.sems is not None
        popped = nc._tile_sem_poison_stack.pop()
        assert popped is self._sem_poison
        nc.clear_and_free_semaphores(list(self.sems.allocated().values()))
        nc.all_engine_barrier()

    tile.TileContext._drain_and_barrier = _patched_drain_and_barrier
    tile.TileContext._drain_patch_installed = True


def _split_sync_waits(nc, max_waits=1):
    from concourse import mybir

    sync_info_cls = None
    for f in nc.m.functions:
        for bb in f.blocks:
            insts = bb.instructions
            out = []
            changed = False
            for inst in insts:
                si = inst.sync_info
                waits = list(si.on_wait) if (si is not None and si.on_wait) else []
                if len(waits) > max_waits:
                    changed = True
                    if sync_info_cls is None:
                        sync_info_cls = type(si)
                    head, tail = waits[:-max_waits], waits[-max_waits:]
                    for k, w in enumerate(head):
                        nop = mybir.InstNoOp(name=f"{inst.name}-sw{k}", ins=[], outs=[])
                        nop.engine = inst.engine
                        nop.sync_info = sync_info_cls(on_wait=[w], on_update=[])
                        out.append(nop)
                    si.on_wait = tail
                out.append(inst)
            if changed:
                bb.instructions = out


# ------------------------------------------------------------- bass program
def _build_bass():
    import concourse.bass as bass
    import concourse.tile as tile
    from concourse import mybir
    from concourse.masks import make_identity

    F32 = mybir.dt.float32
    F32R = mybir.dt.float32r
    BF = mybir.dt.bfloat16
    AF = mybir.ActivationFunctionType
    ALU = mybir.AluOpType
    AX = mybir.AxisListType  # noqa: F841

    _install_tile_patches()
    nc = bass.Bass()

    dp = nc.declare_dram_parameter

    # ---- per-core inputs -------------------------------------------------
    enc_flat = dp("enc_flat", [BP, ENC], BF, isOutput=False)       # (b,p) x enc
    encT = dp("encT", [ENC, BP], BF, isOutput=False)               # enc x (b,p)
    embT = dp("embT", [E, BT], F32R, isOutput=False)               # e x bt
    h0T = dp("h0T", [H, Bc], BF, isOutput=False)
    h0b = dp("h0b", [Bc, H], F32, isOutput=False)
    c0b = dp("c0b", [Bc, H], F32, isOutput=False)
    mask_cols = dp("mask_cols", [Bc, T], F32, isOutput=False)
    mask_bt = dp("mask_bt", [BT, 1], F32, isOutput=False)
    sk_mask = dp("sk_mask", [KP * 128, Bc], BF, isOutput=False)    # seg->sample masks
    # weights (pre-transposed to [in, out] on host)
    w_ea = dp("w_ea", [ENC, A], BF, isOutput=False)
    w_da = dp("w_da", [H, A], BF, isOutput=False)
    b_da = dp("b_da", [A, 1], F32, isOutput=False)                 # dec_att_b + enc_att_b
    w_full = dp("w_full", [A, 1], BF, isOutput=False)
    w_fbeta = dp("w_fbeta", [H, ENC], BF, isOutput=False)
    b_fbeta = dp("b_fbeta", [1, ENC], BF, isOutput=False)
    w_hh = dp("w_hh", [H, H4], BF, isOutput=False)
    w_ihe = dp("w_ihe", [E, H4], F32R, isOutput=False)
    gates_bias = dp("gates_bias", [1, H4], F32R, isOutput=False)   # b_ih + b_hh
    w_ihz = dp("w_ihz", [ENC, H4], BF, isOutput=False)
    w_lh = dp("w_lh", [H, E], BF, isOutput=False)
    w_lz = dp("w_lz", [ENC, E], BF, isOutput=False)
    w_fc = dp("w_fc", [E, V], F32R, isOutput=False)
    e_badd_T = dp("e_badd_T", [E, BT], F32, isOutput=False)        # e_t + lh_b + lz_b

    # ---- outputs ---------------------------------------------------------
    preds_out = dp("preds_out", [BT, V], F32, isOutput=True)
    exp_out = dp("exp_out", [T, BP], BF, isOutput=True)
    sums_out = dp("sums_out", [T, Bc], F32, isOutput=True)

    # ---- internal DRAM scratch ------------------------------------------
    gates_e_seq = nc.dram_tensor("gates_e_seq", [BT, H4], BF)
    h_seqT = nc.dram_tensor("h_seqT", [H, BT], BF)
    z_seqT = nc.dram_tensor("z_seqT", [ENC, BT], BF)

    KE = ENC // 128           # 16 k-tiles over enc dim
    KH = H // 128             # 4 k-tiles over hidden dim
    MT = [128] * (BT // 128) + ([BT % 128] if BT % 128 else [])   # bt m-tiles
    NB = 256                  # att1 n-block
    n_blocks = [NB] * (BP // NB) + ([BP % NB] if BP % NB else [])

    with tile.TileContext(nc) as tc:
        with tc.tile_pool(name="persist", bufs=1) as pp:
            ident = pp.tile([128, 128], F32, tag="ident")
            make_identity(nc, ident[:])
            ident_bf = pp.tile([128, 128], BF, tag="ident_bf")
            make_identity(nc, ident_bf[:])
            ones_f = pp.tile([1, 128], F32, tag="ones_f")
            nc.gpsimd.memset(ones_f[:], 1.0)
            ones_r = pp.tile([1, 128], F32R, tag="ones_r")
            nc.vector.tensor_copy(ones_r[:], ones_f[:])
            ones_bf = pp.tile([1, Bc], BF, tag="ones_bf")
            nc.gpsimd.memset(ones_bf[:], 1.0)

            mask_sb = pp.tile([Bc, T], F32, tag="mask_sb")
            nc.sync.dma_start(mask_sb[:], mask_cols[:])

            att1T = [pp.tile([128, BP], BF, name=f"att1T{m}", tag=f"att1T{m}") for m in range(KH)]
            w_da_sb = [pp.tile([128, A], BF, name=f"wda{k}", tag=f"wda{k}") for k in range(KH)]
            b_da_sb = [pp.tile([128, 1], F32, name=f"bda{k}", tag=f"bda{k}") for k in range(KH)]
            w_full_sb = [pp.tile([128, 1], BF, name=f"wfull{k}", tag=f"wfull{k}") for k in range(KH)]
            w_fbeta_sb = [pp.tile([128, ENC], BF, name=f"wfb{k}", tag=f"wfb{k}") for k in range(KH)]
            b_fbeta_sb = pp.tile([1, ENC], BF, tag="bfb")
            w_hh_sb = [pp.tile([128, H4], BF, name=f"whh{k}", tag=f"whh{k}") for k in range(KH)]
            sk_sb = [pp.tile([128, Bc], BF, name=f"sk{k}", tag=f"sk{k}") for k in range(KP)]
            for k in range(KH):
                nc.sync.dma_start(w_da_sb[k][:], w_da[128 * k:128 * (k + 1), :])
                nc.sync.dma_start(b_da_sb[k][:], b_da[128 * k:128 * (k + 1), :])
                nc.sync.dma_start(w_full_sb[k][:], w_full[128 * k:128 * (k + 1), :])
                nc.sync.dma_start(w_fbeta_sb[k][:], w_fbeta[128 * k:128 * (k + 1), :])
                nc.sync.dma_start(w_hh_sb[k][:], w_hh[128 * k:128 * (k + 1), :])
            nc.sync.dma_start(b_fbeta_sb[:], b_fbeta[:])
            for k in range(KP):
                nc.sync.dma_start(sk_sb[k][:], sk_mask[128 * k:128 * (k + 1), :])

            # ---------------- P0: att1T = (enc @ enc_att_w.T) in aT layout
            with tc.tile_pool(name="p0", bufs=1) as p0, \
                 tc.tile_pool(name="p0enc", bufs=2) as p0enc, \
                 tc.tile_pool(name="p0ps", bufs=4, space="PSUM") as p0ps:
                w_ea_sb = [p0.tile([128, A], BF, name=f"wea{k}", tag=f"wea{k}") for k in range(KE)]
                for k in range(KE):
                    nc.sync.dma_start(w_ea_sb[k][:], w_ea[128 * k:128 * (k + 1), :])
                col = 0
                for blk in n_blocks:
                    echunks = []
                    for k in range(KE):
                        ec = p0enc.tile([128, NB], BF, name=f"e{k}", tag=f"e{k}")
                        nc.sync.dma_start(
                            ec[:, 0:blk], encT[128 * k:128 * (k + 1), col:col + blk]
                        )
                        echunks.append(ec)
                    for m in range(KH):
                        ps = p0ps.tile([128, NB], F32, tag="ps")
                        for k in range(KE):
                            nc.tensor.matmul(
                                ps[:, 0:blk],
                                w_ea_sb[k][:, 128 * m:128 * (m + 1)],
                                echunks[k][:, 0:blk],
                                start=(k == 0), stop=(k == KE - 1),
                            )
                        if m % 2 == 0:
                            nc.vector.tensor_copy(
                                att1T[m][:, col:col + blk], ps[:, 0:blk]
                            )
                        else:
                            nc.scalar.copy(att1T[m][:, col:col + blk], ps[:, 0:blk])
                    col += blk

            # ---------------- P1: gates_e_seq = embT.T @ w_ihe + biases
            with tc.tile_pool(name="p1", bufs=1) as p1, \
                 tc.tile_pool(name="p1w", bufs=2) as p1w, \
                 tc.tile_pool(name="p1ps", bufs=3, space="PSUM") as p1ps:
                embT_sb = [p1.tile([128, BT], F32R, name=f"embT{k}", tag=f"embT{k}") for k in range(KH)]
                for k in range(KH):
                    nc.sync.dma_start(embT_sb[k][:], embT[128 * k:128 * (k + 1), :])
                gb_sb = p1.tile([1, H4], F32R, tag="gb")
                nc.sync.dma_start(gb_sb[:], gates_bias[:])
                w_ihe_sb = [p1.tile([128, H4], F32R, name=f"wihe{k}", tag=f"wihe{k}") for k in range(KH)]
                for k in range(KH):
                    nc.sync.dma_start(w_ihe_sb[k][:], w_ihe[128 * k:128 * (k + 1), :])
                row = 0
                for mrows in MT:
                    for c in range(H4 // 512):
                        ps = p1ps.tile([128, 512], F32, tag="ps")
                        for k in range(KH):
                            nc.tensor.matmul(
                                ps[0:mrows, :],
                                embT_sb[k][:, row:row + mrows],
                                w_ihe_sb[k][:, 512 * c:512 * (c + 1)],
                                start=(k == 0), stop=False,
                            )
                        nc.tensor.matmul(
                            ps[0:mrows, :],
                            ones_r[:, 0:mrows],
                            gb_sb[:, 512 * c:512 * (c + 1)],
                            start=False, stop=True,
                        )
                        ge = p1w.tile([128, 512], BF, tag="ge")
                        nc.vector.tensor_copy(ge[0:mrows, :], ps[0:mrows, :])
                        nc.sync.dma_start(
                            gates_e_seq[row:row + mrows, 512 * c:512 * (c + 1)],
                            ge[0:mrows, :],
                        )
                    row += mrows

            # ---------------- P2: the 51-step recurrent loop
            with tc.tile_pool(name="lw", bufs=1) as lw, \
                 tc.tile_pool(name="st", bufs=2) as st, \
                 tc.tile_pool(name="wk", bufs=2) as wk, \
                 tc.tile_pool(name="enc_pool", bufs=2) as encp, \
                 tc.tile_pool(name="relu_pool", bufs=2) as relup, \
                 tc.tile_pool(name="seg_pool", bufs=3) as segp, \
                 tc.tile_pool(name="ps_big", bufs=1, space="PSUM") as psb, \
                 tc.tile_pool(name="ps_small", bufs=2, space="PSUM") as pss, \
                 tc.tile_pool(name="ps_score", bufs=2, space="PSUM") as psc:
                w_ihz_sb = [lw.tile([128, H4], BF, name=f"wihz{k}", tag=f"wihz{k}") for k in range(KE)]
                for k in range(KE):
                    nc.sync.dma_start(w_ihz_sb[k][:], w_ihz[128 * k:128 * (k + 1), :])

                cur_h = st.tile([Bc, H], F32, tag="h")
                nc.sync.dma_start(cur_h[:], h0b[:])
                cur_c = st.tile([Bc, H], F32, tag="c")
                nc.sync.dma_start(cur_c[:], c0b[:])
                cur_hT = []
                for k in range(KH):
                    t_ = st.tile([128, Bc], BF, tag=f"hT{k}")
                    nc.sync.dma_start(t_[:], h0T[128 * k:128 * (k + 1), :])
                    cur_hT.append(t_)

                for t in range(T):
                    # gates_e prefetch
                    ge_t = wk.tile([Bc, H4], BF, tag="ge_t", bufs=1)
                    nc.sync.dma_start(ge_t[:], gates_e_seq[Bc * t:Bc * (t + 1), :])

                    # att2 = dec_att(h) in aT layout, + combined bias
                    ps_a2 = pss.tile([128, 64], F32, tag="pss")
                    for m in range(KH):
                        for k in range(KH):
                            nc.tensor.matmul(
                                ps_a2[:, 16 * m:16 * (m + 1)],
                                w_da_sb[k][:, 128 * m:128 * (m + 1)],
                                cur_hT[k][:],
                                start=(k == 0), stop=(k == KH - 1),
                            )
                    att2_sb = []
                    for m in range(KH):
                        a2 = wk.tile([128, Bc], F32, name=f"att2_{m}", tag=f"att2_{m}")
                        nc.scalar.activation(
                            a2[:], ps_a2[:, 16 * m:16 * (m + 1)],
                            AF.Identity, bias=b_da_sb[m][:],
                        )
                        att2_sb.append(a2)

                    # beta gate (bf16), psum shared slot "big"
                    ps_beta = psb.tile([Bc, H4], F32, tag="big")
                    for c in range(4):
                        for k in range(KH):
                            nc.tensor.matmul(
                                ps_beta[:, 512 * c:512 * (c + 1)],
                                cur_hT[k][:],
                                w_fbeta_sb[k][:, 512 * c:512 * (c + 1)],
                                start=(k == 0), stop=False,
                            )
                        nc.tensor.matmul(
                            ps_beta[:, 512 * c:512 * (c + 1)],
                            ones_bf[:],
                            b_fbeta_sb[:, 512 * c:512 * (c + 1)],
                            start=False, stop=True,
                        )
                    sigm = wk.tile([Bc, H4], BF, tag="sigm", bufs=1)
                    nc.scalar.activation(sigm[:], ps_beta[:], AF.Sigmoid)

                    # relu(att1+att2) -> score -> exp (per sample)
                    exp_row = wk.tile([1, BP], BF, tag="exp_row", bufs=1)
                    sums_row = wk.tile([1, Bc], F32, tag="sums_row")
                    for b in range(Bc):
                        rel = []
                        for m in range(KH):
                            r_ = relup.tile([128, P], BF, name=f"relu{m}", tag=f"relu{m}")
                            if (b + m) % 2 == 0:
                                nc.scalar.activation(
                                    r_[:], att1T[m][:, P * b:P * (b + 1)],
                                    AF.Relu, bias=att2_sb[m][:, b:b + 1],
                                )
                            else:
                                nc.vector.tensor_scalar(
                                    r_[:], att1T[m][:, P * b:P * (b + 1)],
                                    att2_sb[m][:, b:b + 1], 0.0, ALU.add, ALU.max,
                                )
                            rel.append(r_)
                        ps_sc = psc.tile([1, P], F32, tag="sc")
                        for m in range(KH):
                            nc.tensor.matmul(
                                ps_sc[:], w_full_sb[m][:], rel[m][:],
                                start=(m == 0), stop=(m == KH - 1),
                            )
                        nc.scalar.activation(
                            exp_row[0:1, P * b:P * (b + 1)], ps_sc[:], AF.Exp,
                            accum_out=sums_row[0:1, b:b + 1],
                        )
                    nc.sync.dma_start(exp_out[t:t + 1, :], exp_row[:])
                    nc.sync.dma_start(sums_out[t:t + 1, :], sums_row[:])

                    # 1/sums as a column
                    rec_row = wk.tile([1, Bc], F32, tag="rec_row")
                    nc.vector.reciprocal(rec_row[:], sums_row[:])
                    ps_rt = pss.tile([Bc, 1], F32, tag="pss")
                    nc.tensor.transpose(ps_rt[:], rec_row[:], ident[0:1, 0:1])
                    recT = wk.tile([Bc, 1], F32, tag="recT")
                    nc.vector.tensor_copy(recT[:], ps_rt[:])

                    # z (unnormalized): block-diagonal exp weights @ enc
                    ps_z = psb.tile([Bc, ENC], F32, tag="big")
                    for k in range(KP):
                        rk = min(128, BP - 128 * k)
                        ps_seg = pss.tile([128, 1], BF, tag="pss")
                        nc.tensor.transpose(
                            ps_seg[0:rk, :], exp_row[0:1, 128 * k:128 * k + rk],
                            ident_bf[0:1, 0:1],
                        )
                        seg = segp.tile([128, 1], F32, tag="seg")
                        nc.vector.tensor_copy(seg[0:rk, :], ps_seg[0:rk, :])
                        ext = segp.tile([128, Bc], BF, tag="ext")
                        nc.vector.tensor_scalar(
                            ext[0:rk, :], sk_sb[k][0:rk, :], seg[0:rk, :], None,
                            ALU.mult,
                        )
                        enc_k = encp.tile([128, ENC], BF, tag="enc_k")
                        nc.sync.dma_start(
                            enc_k[0:rk, :], enc_flat[128 * k:128 * k + rk, :]
                        )
                        for c in range(4):
                            nc.tensor.matmul(
                                ps_z[:, 512 * c:512 * (c + 1)],
                                ext[0:rk, :],
                                enc_k[0:rk, 512 * c:512 * (c + 1)],
                                start=(k == 0), stop=(k == KP - 1),
                            )

                    # z_gated = sigmoid(beta) * recip * z
                    zg = wk.tile([Bc, ENC], BF, tag="zg", bufs=1)
                    nc.vector.scalar_tensor_tensor(
                        out=zg[:], in0=sigm[:], scalar=recT[:], in1=ps_z[:],
                        op0=ALU.mult, op1=ALU.mult,
                    )

                    # gates: hh part first, then W_ihz via zT transposes
                    ps_g = psb.tile([Bc, H4], F32, tag="big")
                    for c in range(4):
                        for k in range(KH):
                            nc.tensor.matmul(
                                ps_g[:, 512 * c:512 * (c + 1)],
                                cur_hT[k][:],
                                w_hh_sb[k][:, 512 * c:512 * (c + 1)],
                                start=(k == 0), stop=False,
                            )
                    for k2 in range(KE):
                        ps_t = pss.tile([128, Bc], BF, tag="pss")
                        nc.tensor.transpose(
                            ps_t[:], zg[:, 128 * k2:128 * (k2 + 1)],
                            ident_bf[0:Bc, 0:Bc],
                        )
                        zTt = segp.tile([128, Bc], BF, tag="zTt")
                        nc.vector.tensor_copy(zTt[:], ps_t[:])
                        nc.sync.dma_start(
                            z_seqT[128 * k2:128 * (k2 + 1), Bc * t:Bc * (t + 1)],
                            zTt[:],
                        )
                        for c in range(4):
                            nc.tensor.matmul(
                                ps_g[:, 512 * c:512 * (c + 1)],
                                zTt[:],
                                w_ihz_sb[k2][:, 512 * c:512 * (c + 1)],
                                start=False, stop=(k2 == KE - 1),
                            )
                    gates = wk.tile([Bc, H4], F32, tag="gates", bufs=1)
                    nc.vector.tensor_tensor(gates[:], ps_g[:], ge_t[:], ALU.add)

                    # LSTM pointwise (tiles reused to cap SBUF)
                    gi = wk.tile([Bc, H], F32, tag="gi", bufs=1)
                    nc.scalar.activation(gi[:], gates[:, 0:H], AF.Sigmoid)
                    gf = wk.tile([Bc, H], F32, tag="gf", bufs=1)
                    nc.scalar.activation(gf[:], gates[:, H:2 * H], AF.Sigmoid)
                    gg = wk.tile([Bc, H], F32, tag="gg", bufs=1)
                    nc.scalar.activation(gg[:], gates[:, 2 * H:3 * H], AF.Tanh)
                    go = wk.tile([Bc, H], F32, tag="go", bufs=1)
                    nc.scalar.activation(go[:], gates[:, 3 * H:4 * H], AF.Sigmoid)
                    cn = wk.tile([Bc, H], F32, tag="cn", bufs=1)
                    nc.vector.tensor_tensor(cn[:], gf[:], cur_c[:], ALU.mult)
                    hn = wk.tile([Bc, H], F32, tag="hn", bufs=1)
                    nc.vector.tensor_tensor(hn[:], gi[:], gg[:], ALU.mult)
                    nc.vector.tensor_tensor(cn[:], cn[:], hn[:], ALU.add)
                    nc.scalar.activation(gg[:], cn[:], AF.Tanh)
                    nc.vector.tensor_tensor(hn[:], go[:], gg[:], ALU.mult)

                    m_col = mask_sb[:, t:t + 1]
                    nc.vector.tensor_tensor(gi[:], hn[:], cur_h[:], ALU.subtract)
                    h_nxt = st.tile([Bc, H], F32, tag="h")
                    nc.vector.scalar_tensor_tensor(
                        out=h_nxt[:], in0=gi[:], scalar=m_col, in1=cur_h[:],
                        op0=ALU.mult, op1=ALU.add,
                    )
                    nc.vector.tensor_tensor(gf[:], cn[:], cur_c[:], ALU.subtract)
                    c_nxt = st.tile([Bc, H], F32, tag="c")
                    nc.vector.scalar_tensor_tensor(
                        out=c_nxt[:], in0=gf[:], scalar=m_col, in1=cur_c[:],
                        op0=ALU.mult, op1=ALU.add,
                    )

                    new_hT = []
                    for k in range(KH):
                        ps_t = pss.tile([128, Bc], F32, tag="pss")
                        nc.tensor.transpose(
                            ps_t[:], h_nxt[:, 128 * k:128 * (k + 1)],
                            ident[0:Bc, 0:Bc],
                        )
                        hT_k = st.tile([128, Bc], BF, tag=f"hT{k}")
                        nc.scalar.copy(hT_k[:], ps_t[:])
                        nc.sync.dma_start(
                            h_seqT[128 * k:128 * (k + 1), Bc * t:Bc * (t + 1)],
                            hT_k[:],
                        )
                        new_hT.append(hT_k)
                    cur_h, cur_c, cur_hT = h_nxt, c_nxt, new_hT

            # ---------------- P3: pre = e + lh(h) + lz(z); preds = fc(pre)
            with tc.tile_pool(name="p3", bufs=1) as p3, \
                 tc.tile_pool(name="p3fc", bufs=3) as p3fc, \
                 tc.tile_pool(name="p3o", bufs=4) as p3o, \
                 tc.tile_pool(name="p3ps", bufs=2, space="PSUM") as p3ps, \
                 tc.tile_pool(name="p3psfc", bufs=4, space="PSUM") as p3psfc:
                hseq_sb = [p3.tile([128, BT], BF, name=f"hs{k}", tag=f"hs{k}") for k in range(KH)]
                for k in range(KH):
                    nc.sync.dma_start(hseq_sb[k][:], h_seqT[128 * k:128 * (k + 1), :])
                zseq_sb = [p3.tile([128, BT], BF, name=f"zs{k}", tag=f"zs{k}") for k in range(KE)]
                for k in range(KE):
                    nc.sync.dma_start(zseq_sb[k][:], z_seqT[128 * k:128 * (k + 1), :])
                w_lh_sb = [p3.tile([128, E], BF, name=f"wlh{k}", tag=f"wlh{k}") for k in range(KH)]
                for k in range(KH):
                    nc.sync.dma_start(w_lh_sb[k][:], w_lh[128 * k:128 * (k + 1), :])
                w_lz_sb = [p3.tile([128, E], BF, name=f"wlz{k}", tag=f"wlz{k}") for k in range(KE)]
                for k in range(KE):
                    nc.sync.dma_start(w_lz_sb[k][:], w_lz[128 * k:128 * (k + 1), :])
                ebadd_sb = [p3.tile([128, BT], F32, name=f"eb{k}", tag=f"eb{k}") for k in range(KH)]
                for k in range(KH):
                    nc.sync.dma_start(ebadd_sb[k][:], e_badd_T[128 * k:128 * (k + 1), :])
                mb_sb = []
                rr = 0
                for i, mrows in enumerate(MT):
                    mb = p3.tile([128, 1], F32, tag=f"mb{i}")
                    nc.sync.dma_start(mb[0:mrows, :], mask_bt[rr:rr + mrows, :])
                    mb_sb.append(mb)
                    rr += mrows

                preT = [p3.tile([128, BT], F32R, name=f"preT{m}", tag=f"preT{m}") for m in range(KH)]
                for m in range(KH):
                    for (c0, cw) in ((0, 512), (512, 304)):
                        ps = p3ps.tile([128, 512], F32, tag="ps")
                        for k in range(KH):
                            nc.tensor.matmul(
                                ps[:, 0:cw],
                                w_lh_sb[k][:, 128 * m:128 * (m + 1)],
                                hseq_sb[k][:, c0:c0 + cw],
                                start=(k == 0), stop=False,
                            )
                        for k in range(KE):
                            nc.tensor.matmul(
                                ps[:, 0:cw],
                                w_lz_sb[k][:, 128 * m:128 * (m + 1)],
                                zseq_sb[k][:, c0:c0 + cw],
                                start=False, stop=(k == KE - 1),
                            )
                        nc.vector.tensor_tensor(
                            preT[m][:, c0:c0 + cw], ps[:, 0:cw],
                            ebadd_sb[m][:, c0:c0 + cw], ALU.add,
                        )

                VCH = 500
                n_vch = V // VCH
                for v in range(n_vch):
                    fc_t = p3fc.tile([128, 4 * VCH], F32R, tag="fc_t")
                    for k in range(KH):
                        nc.sync.dma_start(
                            fc_t[:, VCH * k:VCH * (k + 1)],
                            w_fc[128 * k:128 * (k + 1), VCH * v:VCH * (v + 1)],
                        )
                    row = 0
                    for i, mrows in enumerate(MT):
                        ps = p3psfc.tile([128, VCH], F32, tag="ps")
                        for k in range(KH):
                            nc.tensor.matmul(
                                ps[0:mrows, :],
                                preT[k][:, row:row + mrows],
                                fc_t[:, VCH * k:VCH * (k + 1)],
                                start=(k == 0), stop=(k == KH - 1),
                            )
                        po = p3o.tile([128, VCH], F32, tag="po")
                        if i % 2 == 0:
                            nc.scalar.activation(
                                po[0:mrows, :], ps[0:mrows, :], AF.Copy,
                                scale=mb_sb[i][0:mrows, :],
                            )
                        else:
                            nc.vector.tensor_scalar(
                                po[0:mrows, :], ps[0:mrows, :],
                                mb_sb[i][0:mrows, :], None, ALU.mult,
                            )
                        nc.sync.dma_start(
                            preds_out[row:row + mrows, VCH * v:VCH * (v + 1)],
                            po[0:mrows, :],
                        )
                        row += mrows

    _split_sync_waits(nc)
    return nc


def _get_bass():
    if "nc" not in _CACHE:
        _CACHE["nc"] = _build_bass()
    return _CACHE["nc"]


# ------------------------------------------------------------------- driver
def kernel(encoder_out, encoded_captions, caption_lengths, params):
    from concourse.bass_utils import run_bass_kernel_spmd

    encoder_out = np.asarray(encoder_out)
    encoded_captions = np.asarray(encoded_captions)
    caption_lengths = np.asarray(caption_lengths)
    p = {k: np.asarray(v) for k, v in params.items()}

    lens = caption_lengths[:, 0]
    sort_ind = np.argsort(-lens, kind="stable")
    enc_s = encoder_out[sort_ind]
    caps_s = encoded_captions[sort_ind]
    lens_s = lens[sort_ind]
    dl = lens_s - 1                     # decode lengths

    f32 = np.float32
    emb = p["emb"].astype(f32)
    lh_b, lz_b = p["lh_b"].astype(f32), p["lz_b"].astype(f32)

    # shared (replicated) weight tensors
    shared = {
        "w_ea": np.ascontiguousarray(p["enc_att_w"].T).astype(BF16),
        "w_da": np.ascontiguousarray(p["dec_att_w"].T).astype(BF16),
        "b_da": (p["dec_att_b"] + p["enc_att_b"]).astype(f32).reshape(A, 1),
        "w_full": p["full_att_w"][0].astype(BF16).reshape(A, 1),
        "w_fbeta": np.ascontiguousarray(p["f_beta_w"].T).astype(BF16),
        "b_fbeta": p["f_beta_b"].astype(BF16).reshape(1, ENC),
        "w_hh": np.ascontiguousarray(p["lstm_w_hh"].T).astype(BF16),
        "w_ihe": np.ascontiguousarray(p["lstm_w_ih"][:, :E].T).astype(f32),
        "gates_bias": (p["lstm_b_ih"] + p["lstm_b_hh"]).astype(f32).reshape(1, H4),
        "w_ihz": np.ascontiguousarray(p["lstm_w_ih"][:, E:].T).astype(BF16),
        "w_lh": np.ascontiguousarray(p["lh_w"].T).astype(BF16),
        "w_lz": np.ascontiguousarray(p["lz_w"].T).astype(BF16),
        "w_fc": np.ascontiguousarray(p["fc_w"].T).astype(f32),
    }

    # seg -> sample one-hot masks for the block-diagonal z matmul
    g = np.arange(KP * 128)
    sk = np.zeros((KP * 128, Bc), np.float32)
    valid = g < BP
    sk[valid, np.minimum(g[valid] // P, Bc - 1)] = 1.0
    shared["sk_mask"] = sk.astype(BF16)

    in_maps = []
    for c in range(NCORES):
        sl = slice(Bc * c, Bc * (c + 1))
        enc_c = enc_s[sl].astype(f32)                        # [16,196,2048]
        enc_flat = enc_c.reshape(BP, ENC)
        mean = enc_c.mean(axis=1)                            # [16, 2048]
        h0 = mean @ p["init_h_w"].T.astype(f32) + p["init_h_b"].astype(f32)
        c0 = mean @ p["init_c_w"].T.astype(f32) + p["init_c_b"].astype(f32)
        emb_seq = emb[caps_s[sl, :T]]                        # [16, 51, 512]
        embT = np.ascontiguousarray(emb_seq.transpose(2, 1, 0).reshape(E, BT))
        e_badd = embT + (lh_b + lz_b)[:, None]
        dlc = dl[sl]
        mask = (np.arange(T)[None, :] < dlc[:, None]).astype(f32)   # [16, 51]
        m = {
            "enc_flat": enc_flat.astype(BF16),
            "encT": np.ascontiguousarray(enc_flat.T).astype(BF16),
            "embT": embT.astype(f32),
            "h0T": np.ascontiguousarray(h0.T).astype(BF16),
            "h0b": h0.astype(f32),
            "c0b": c0.astype(f32),
            "mask_cols": mask,
            "mask_bt": np.ascontiguousarray(mask.T).reshape(BT, 1).astype(f32),
            "e_badd_T": e_badd.astype(f32),
        }
        m.update(shared)
        in_maps.append(m)

    nc = _get_bass()
    global LAST_RESULT
    LAST_RESULT = run_bass_kernel_spmd(
        nc, in_maps, list(range(NCORES)), trace=TRACE
    )
    res = LAST_RESULT.results

    preds = np.empty((B, T, V), f32)
    alphas = np.empty((B, T, P), f32)
    for c in range(NCORES):
        sl = slice(Bc * c, Bc * (c + 1))
        pr = res[c]["preds_out"].reshape(T, Bc, V)
        preds[sl] = pr.transpose(1, 0, 2)
        ex = res[c]["exp_out"].astype(f32).reshape(T, Bc, P)  # [51, 16, 196]
        sm = res[c]["sums_out"].reshape(T, Bc, 1)
        al = ex / sm                                         # softmax
        mask = (np.arange(T)[None, :] < dl[sl][:, None])
        alphas[sl] = al.transpose(1, 0, 2) * mask[:, :, None]

    if np.any(p["fc_b"]):
        fcb = p["fc_b"].astype(f32)[None, None, :]
        mask_all = (np.arange(T)[None, :] < dl[:, None])
        preds = preds + fcb * mask_all[:, :, None]

    ind_dt = np.int32 if caption_lengths.dtype.itemsize == 4 else np.int64
    return preds, caps_s, alphas, sort_ind.astype(ind_dt)
